# revision 29
# baseline (speedup 1.0000x reference)
"""CoAttention kernel for Trainium2, 8 NeuronCores, pure data parallel.

Math shortcut (exact, from softmax shift-invariance): in the reference,
scores1[b,s,r] = A[b,s] + C[b,r] + const, and softmax is over r, so the
attention weights are independent of s:
    visual_att[b,s,:] = softmax_r(tanh(img[b] @ Wi1) @ wa1[D:])
    att_img_features[b,s,:] = p[b] @ img[b]            (same row for all s)
Likewise stage 2's textual_att is independent of the query index i:
    textual_att[b,i,:] = softmax_j(tanh(text[b] @ Wt2) @ wa2[D:])
    att_text_features[b,i,:] = q[b] @ text[b]          (same row for all i)
Wt1/bt1/Wi2/bi2/wa1[:D]/wa2[:D]/ba1/ba2 cancel exactly.

Each core handles B/8 = 4 batches and outputs the per-batch vectors
u[b] (text) and v[b] (img); the host broadcasts them over S.

Design:
- The heavy X@W matmuls run in fp8 e4m3 with MatmulPerfMode.DoubleRow
  (256-deep contraction per instruction, half the per-row PE cost of
  bf16).  Weights are pre-scaled by 64 on the host so fp8 stays out of
  the denormal range; the 1/64 is folded into the tanh/exp activation
  scale (exact powers of two).
- X^T is pre-transposed on the host and shipped as fp8 (no DMA xbar
  transposes).  X natural stays bf16 — it feeds the weighted sums that
  produce the outputs directly, where fp8 would eat the error budget.
- d = w.tanh(XW) is computed directly in TRANSPOSED form: t2t (fp8) is
  the matmul stationary operand and the 64w column the 1-wide moving
  operand, so [128 tokens, 1] d columns accumulate straight into a
  PSUM dcol tile (1-cycle matmuls, no copies or transposes), emitted
  eagerly after the tanh of each ntile pair.
- Per stage: ONE exp activation over all token-tile columns; softmax
  denominators via a stationary-qcol column-sum matmul, a tiny
  selector matmul (whose extra constant row exactly subtracts the img
  zero-padding's exp(0)=1 terms), one DVE reciprocal.
- Weighted sums use X-natural as the matmul *stationary* operand with a
  single q column as the moving operand (1-cycle matmuls), accumulated
  per output column in one PSUM bank; normalization is applied by a
  DVE per-partition tensor_scalar multiply after a PE transpose.
- The img stage's finish chain is emitted between the two text groups
  so it overlaps text compute (ACT stays tanh/exp-only and in order);
  outputs go out over HWDGE as two [24, 128] row blocks.
- The ACT engine paces the kernel (gapless tanh stream), so one tanh
  group per stage runs on the otherwise-idle DVE as a Pade [2/2]
  rational approximation, and xt_t is DMA'd in two token halves so the
  text matmuls start as soon as group 0's tokens land.
"""

import numpy as np
import ml_dtypes

import concourse.bacc as bacc
import concourse.mybir as mybir
import concourse.tile as tile
from concourse.bass_utils import run_bass_kernel_spmd
from concourse.masks import make_identity

B, S, R, D = 32, 512, 196, 768
NCORES = 8
BPC = B // NCORES          # batches per core
P = 128
KT = D // P                # 6 contraction tiles
NT = D // P                # 6 output-feature tiles
KP = KT // 2               # 3 DoubleRow contraction pairs
RPAD = 256                 # img tokens padded to 2 tiles per batch
TTOK = BPC * S             # 2048 text tokens per core
ITOK = BPC * RPAD          # 1024 padded img tokens per core
GRP = 1024                 # tokens per tanh group (2 PSUM banks)
CH = 256                   # tokens per DoubleRow matmul chunk
NCH = GRP // CH            # 4 chunks per group
WS = 64.0                  # host-side weight pre-scale (exact power of 2)
F32 = mybir.dt.float32
BF16 = mybir.dt.bfloat16
FP8 = mybir.dt.float8e4
AF = mybir.ActivationFunctionType
DR = mybir.MatmulPerfMode.DoubleRow

NP_FP8 = ml_dtypes.float8_e4m3
NP_BF16 = ml_dtypes.bfloat16

_CACHE = {}

# schedule-tuning knobs
CFG = {
    "pm_bufs": 3,          # [128,1024] f32 = 2 banks each
    "sm_bufs": 1,          # rotating small psum tiles
}


def _sel_matrices():
    """Per-stage selector matrices mapping per-token-tile exp sums to the
    24 (batch, dtile) output rows.  sel_i row 8 pairs with the constant
    1/128 qcol column to subtract the RPAD-R zero-pad tokens' exp(0)=1
    contributions from each img batch sum."""
    sel_t = np.zeros((16, 24), np.float32)
    sel_i = np.zeros((9, 24), np.float32)
    for m in range(24):
        b = m // 6
        for i in range(4):
            sel_t[4 * b + i, m] = 1.0
        sel_i[2 * b, m] = 1.0
        sel_i[2 * b + 1, m] = 1.0
        sel_i[8, m] = -(RPAD - R)
    return sel_t, sel_i


def _build():
    nc = bacc.Bacc("TRN2", target_bir_lowering=False, debug=False,
                   num_devices=NCORES)
    dr = {}
    dr["xt_t"] = nc.dram_tensor("xt_t", [P, KT, TTOK], FP8,
                                kind="ExternalInput").ap()
    dr["xt_i"] = nc.dram_tensor("xt_i", [P, KT, ITOK], FP8,
                                kind="ExternalInput").ap()
    dr["nat_t"] = nc.dram_tensor("nat_t", [P, TTOK // P, D], BF16,
                                 kind="ExternalInput").ap()
    dr["nat_i"] = nc.dram_tensor("nat_i", [P, ITOK // P, D], BF16,
                                 kind="ExternalInput").ap()
    dr["w1"] = nc.dram_tensor("w1", [P, KT, D], FP8,
                              kind="ExternalInput").ap()
    dr["w2"] = nc.dram_tensor("w2", [P, KT, D], FP8,
                              kind="ExternalInput").ap()
    dr["wc1"] = nc.dram_tensor("wc1", [P, KT, 1], FP8,
                               kind="ExternalInput").ap()
    dr["wc2"] = nc.dram_tensor("wc2", [P, KT, 1], FP8,
                               kind="ExternalInput").ap()
    dr["sel_t"] = nc.dram_tensor("sel_t", [16, 24], F32,
                                 kind="ExternalInput").ap()
    dr["sel_i"] = nc.dram_tensor("sel_i", [9, 24], F32,
                                 kind="ExternalInput").ap()
    dr["u_out"] = nc.dram_tensor("u_out", [BPC, D], F32,
                                 kind="ExternalOutput").ap()
    dr["v_out"] = nc.dram_tensor("v_out", [BPC, D], F32,
                                 kind="ExternalOutput").ap()

    with tile.TileContext(nc) as tc:
        _emit(tc, dr)
    nc.compile()
    return nc


def _emit(tc, dr):
    from contextlib import ExitStack

    nc = tc.nc
    with ExitStack() as ctx:
        const = ctx.enter_context(tc.tile_pool(name="const", bufs=1))
        xpool = ctx.enter_context(tc.tile_pool(name="x", bufs=1))
        spool = ctx.enter_context(tc.tile_pool(name="small", bufs=1))
        pm_ps = ctx.enter_context(
            tc.tile_pool(name="pm", bufs=CFG["pm_bufs"], space="PSUM"))
        sm_ps = ctx.enter_context(
            tc.tile_pool(name="sm", bufs=CFG["sm_bufs"], space="PSUM"))
        pade = ctx.enter_context(tc.tile_pool(name="pade", bufs=2))
        _emit_body(tc, const, xpool, spool, pm_ps, sm_ps, pade, dr)


def _emit_body(tc, const, xpool, spool, pm_ps, sm_ps, pade, dr):
    nc = tc.nc

    # ---- input DMAs (all pre-laid-out by the host; order = need order;
    # w1/xt_i split by k-pair: the early partial-k matmuls double as the
    # PE frequency-ramp warmup) ----
    w1 = xpool.tile([P, KT, D], FP8)
    xt_i = xpool.tile([P, KT, ITOK], FP8)
    for kp in range(KP):
        nc.sync.dma_start(w1[:, 2 * kp:2 * kp + 2, :],
                          dr["w1"][:, 2 * kp:2 * kp + 2, :])
        nc.sync.dma_start(xt_i[:, 2 * kp:2 * kp + 2, :],
                          dr["xt_i"][:, 2 * kp:2 * kp + 2, :])
    w2 = xpool.tile([P, KT, D], FP8)
    nc.sync.dma_start(w2[:], dr["w2"][:])
    xt_t = xpool.tile([P, KT, TTOK], FP8)
    for g in range(TTOK // GRP):
        nc.sync.dma_start(xt_t[:, :, g * GRP:(g + 1) * GRP],
                          dr["xt_t"][:, :, g * GRP:(g + 1) * GRP])
    wc1 = const.tile([P, KT, 1], FP8)
    nc.sync.dma_start(wc1[:], dr["wc1"][:])
    wc2 = const.tile([P, KT, 1], FP8)
    nc.sync.dma_start(wc2[:], dr["wc2"][:])
    sel_t = const.tile([16, 24], F32)
    nc.sync.dma_start(sel_t[:], dr["sel_t"][:])
    sel_i = const.tile([9, 24], F32)
    nc.sync.dma_start(sel_i[:], dr["sel_i"][:])
    nat_i = xpool.tile([P, ITOK // P, D], BF16)
    nc.sync.dma_start(nat_i[:], dr["nat_i"][:])
    nat_t = xpool.tile([P, TTOK // P, D], BF16)
    nc.sync.dma_start(nat_t[:], dr["nat_t"][:])
    ident = const.tile([P, P], F32)
    make_identity(nc, ident)
    ones = const.tile([P, 1], BF16)
    nc.gpsimd.memset(ones[:], 1.0)

    t2t_i = xpool.tile([P, KT, ITOK], FP8)
    t2t_t = xpool.tile([P, KT, TTOK], FP8)
    dcol_all = sm_ps.tile([P, 24], F32, tag="dcol")
    dcol_t = dcol_all[:, 0:16]
    dcol_i = dcol_all[:, 16:24]

    def dve_tanh(pm, t2t_out):
        """tanh(pm/64) ~ x(27+x^2)/(27+9x^2) (Pade [2/2]) on the mostly
        idle DVE, freeing ~1us of the tanh-saturated ACT engine per use.
        |err| < 0.03 for |x| <= 3 and t2t only feeds the d-score dot, so
        the softmax-logit impact is far below the fp8 noise floor."""
        xc = pade.tile([P, GRP], BF16, tag="pd_xc")
        nc.vector.tensor_scalar_mul(xc[:], pm[:], 1.0 / WS)
        s = pade.tile([P, GRP], BF16, tag="pd_s")
        nc.vector.tensor_mul(s[:], xc[:], xc[:])
        num = pade.tile([P, GRP], BF16, tag="pd_num")
        nc.vector.tensor_scalar_add(num[:], s[:], 27.0)
        den = pade.tile([P, GRP], BF16, tag="pd_den")
        nc.vector.tensor_scalar(den[:], s[:], 9.0, 27.0,
                                mybir.AluOpType.mult, mybir.AluOpType.add)
        r = pade.tile([P, GRP], BF16, tag="pd_r")
        with nc.allow_low_precision(reason="Pade tanh approx: 0.4% bf16 "
                                    "recip noise << 2% Pade error"):
            nc.vector.reciprocal(r[:], den[:])
        m = pade.tile([P, GRP], BF16, tag="pd_m")
        nc.vector.tensor_mul(m[:], xc[:], num[:])
        nc.vector.tensor_mul(t2t_out, m[:], r[:])

    def stage_group(xt, W, wc, t2t, dcol, g, dve_nt=None):
        """One 1024-token group: 6 ntile matmul+tanh rounds (one tanh
        optionally on the DVE); the d-dots are emitted separately."""
        for nt in range(NT):
            pm = pm_ps.tile([P, GRP], F32, tag="pm")
            for ch in range(NCH):
                for kp in range(KP):
                    nc.tensor.matmul(
                        pm[:, CH * ch:CH * (ch + 1)],
                        lhsT=W[:, 2 * kp:2 * kp + 2, nt * P:(nt + 1) * P],
                        rhs=xt[:, 2 * kp:2 * kp + 2,
                               g * GRP + CH * ch:g * GRP + CH * (ch + 1)],
                        start=(kp == 0),
                        stop=(kp == KP - 1),
                        perf_mode=DR,
                    )
            if nt == dve_nt:
                dve_tanh(pm, t2t[:, nt, g * GRP:(g + 1) * GRP])
            else:
                nc.scalar.activation(t2t[:, nt, g * GRP:(g + 1) * GRP],
                                     pm[:, :], AF.Tanh, scale=1.0 / WS)

    def stage_ddot(wc, t2t, dcol, g):
        """d-dot columns for group g.  Each dcol column's start..stop
        accumulation runs to completion before the next column's start:
        a PSUM start marks the whole 2KB zero region pending-zero, so
        interleaving groups in one bank corrupts earlier partials."""
        for tt in range(g * (GRP // P), (g + 1) * (GRP // P)):
            for kp in range(KP):
                nc.tensor.matmul(
                    dcol[:, tt:tt + 1],
                    lhsT=t2t[:, 2 * kp:2 * kp + 2, tt * P:(tt + 1) * P],
                    rhs=wc[:, 2 * kp:2 * kp + 2, :],
                    start=(kp == 0),
                    stop=(kp == KP - 1),
                    perf_mode=DR,
                )

    def stage_exp(dcol, ncol, qcol, pad):
        nc.scalar.activation(qcol[:, 0:ncol], dcol[:, 0:ncol], AF.Exp,
                             scale=1.0 / WS)
        if pad:
            nc.vector.memset(qcol[:, ncol:ncol + 1], 1.0 / P)

    def stage_finish(qcol, nq, sel, nat, tiles_per_b, out_view, tag, ve):
        """ve: engine for the copies/scale (GPSIMD can't touch PSUM, so
        both stages use the DVE; the img chain simply queues after the
        Pade ops, well before its v_out is needed)."""
        # softmax denominators: per-column sums -> selector -> reciprocal
        scol_ps = sm_ps.tile([P, 1], F32, tag="sm", name=f"scol{tag}")
        nc.tensor.matmul(scol_ps[0:nq, :], lhsT=qcol[:, 0:nq],
                         rhs=ones[:, 0:1], start=True, stop=True)
        scol_sb = spool.tile([nq, 1], F32, name=f"scolsb{tag}")
        ve.tensor_copy(scol_sb[:], scol_ps[0:nq, :])
        s24_ps = sm_ps.tile([P, 1], F32, tag="sm", name=f"s24{tag}")
        nc.tensor.matmul(s24_ps[0:24, :], lhsT=sel[0:nq, :],
                         rhs=scol_sb[0:nq, :], start=True, stop=True)
        s24_sb = spool.tile([24, 1], F32, name=f"s24sb{tag}")
        ve.tensor_copy(s24_sb[:], s24_ps[0:24, :])
        rec = spool.tile([24, 1], F32, name=f"rec{tag}")
        nc.vector.reciprocal(rec[:], s24_sb[:])
        # weighted sums: X natural stationary, q column moving (1-cycle)
        ups = sm_ps.tile([P, 24], F32, tag="sm", name=f"ups{tag}")
        for m in range(24):
            b, dt_ = divmod(m, 6)
            tts = [tiles_per_b * b + i for i in range(tiles_per_b)]
            for i, tt in enumerate(tts):
                nc.tensor.matmul(
                    ups[:, m:m + 1],
                    lhsT=nat[:, tt, dt_ * P:(dt_ + 1) * P],
                    rhs=qcol[:, tt:tt + 1],
                    start=(i == 0),
                    stop=(i == len(tts) - 1),
                )
        ut_sb = spool.tile([P, 24], F32, name=f"ut{tag}")
        ve.tensor_copy(ut_sb[:], ups[:])
        o24 = sm_ps.tile([24, P], F32, tag="sm", name=f"o24{tag}")
        nc.tensor.transpose(o24[:], ut_sb[:], ident[:, :])
        osb = spool.tile([24, P], F32, name=f"osb{tag}")
        ve.tensor_scalar_mul(osb[:], o24[0:24, :], rec[:, 0:1])
        nc.sync.dma_start(out_view, osb[0:24, :])

    # ---- pipeline: one tanh per 1024-token group of each stage runs as a
    # Pade chain on the DVE (nt5, so the ACT tanhs for nt0..4 pace on);
    # d-dots that depend on the DVE result are emitted late enough that
    # the in-order PE queue never blocks a tanh-feeding matmul fill.
    u_view = dr["u_out"].rearrange("b (t j) -> (b t) j", j=P)
    v_view = dr["v_out"].rearrange("b (t j) -> (b t) j", j=P)
    qcol_i = spool.tile([P, ITOK // P + 1], BF16)
    qcol_t = spool.tile([P, TTOK // P], BF16)

    stage_group(xt_i, w1, wc1, t2t_i, dcol_i, 0, dve_nt=5)
    stage_group(xt_t, w2, wc2, t2t_t, dcol_t, 0, dve_nt=5)
    stage_ddot(wc1, t2t_i, dcol_i, 0)
    stage_exp(dcol_i, ITOK // P, qcol_i, pad=True)
    stage_group(xt_t, w2, wc2, t2t_t, dcol_t, 1)
    stage_finish(qcol_i, ITOK // P + 1, sel_i, nat_i, 2, v_view, "i",
                 nc.vector)
    stage_ddot(wc2, t2t_t, dcol_t, 0)
    stage_ddot(wc2, t2t_t, dcol_t, 1)
    stage_exp(dcol_t, TTOK // P, qcol_t, pad=False)
    stage_finish(qcol_t, TTOK // P, sel_t, nat_t, 4, u_view, "t",
                 nc.vector)


def _get_nc():
    if "nc" not in _CACHE:
        _CACHE["nc"] = _build()
    return _CACHE["nc"]


def _prep_inputs(inputs):
    """Full inputs -> list of NCORES per-core input dicts (host layouts)."""
    text = np.asarray(inputs["text_features"], np.float32)
    img_raw = np.asarray(inputs["img_features"], np.float32)
    img = np.zeros((B, RPAD, D), np.float32)
    img[:, :R, :] = img_raw

    Wi1 = np.asarray(inputs["Wi1"], np.float32) * WS
    Wt2 = np.asarray(inputs["Wt2"], np.float32) * WS
    w1v = np.asarray(inputs["wa1"], np.float32)[D:] * WS
    w2v = np.asarray(inputs["wa2"], np.float32)[D:] * WS

    # weights: [P, KT, D] fp8 with W[p, j, n] = 64*W[j*128+p, n]
    w1_l = np.ascontiguousarray(
        Wi1.reshape(KT, P, D).transpose(1, 0, 2)).astype(NP_FP8)
    w2_l = np.ascontiguousarray(
        Wt2.reshape(KT, P, D).transpose(1, 0, 2)).astype(NP_FP8)
    wc1_l = np.ascontiguousarray(
        w1v.reshape(KT, P).T.reshape(P, KT, 1)).astype(NP_FP8)
    wc2_l = np.ascontiguousarray(
        w2v.reshape(KT, P).T.reshape(P, KT, 1)).astype(NP_FP8)
    sel_t, sel_i = _sel_matrices()

    in_maps = []
    for c in range(NCORES):
        tc_ = text[BPC * c:BPC * (c + 1)].reshape(TTOK, D)
        ic_ = img[BPC * c:BPC * (c + 1)].reshape(ITOK, D)
        # X^T: [P, KT, TOK] with A[p, j, t] = X[t, j*128+p]
        xt_t = np.ascontiguousarray(
            tc_.T.reshape(KT, P, TTOK).transpose(1, 0, 2)).astype(NP_FP8)
        xt_i = np.ascontiguousarray(
            ic_.T.reshape(KT, P, ITOK).transpose(1, 0, 2)).astype(NP_FP8)
        # X natural: [P, TOK//P, D] with B[p, t, n] = X[t*128+p, n]
        nat_t = np.ascontiguousarray(
            tc_.reshape(TTOK // P, P, D).transpose(1, 0, 2)).astype(NP_BF16)
        nat_i = np.ascontiguousarray(
            ic_.reshape(ITOK // P, P, D).transpose(1, 0, 2)).astype(NP_BF16)
        in_maps.append({
            "xt_t": xt_t, "xt_i": xt_i, "nat_t": nat_t, "nat_i": nat_i,
            "w1": w1_l, "w2": w2_l, "wc1": wc1_l, "wc2": wc2_l,
            "sel_t": sel_t, "sel_i": sel_i,
        })
    return in_maps


def kernel(**inputs):
    nc = _get_nc()
    in_maps = _prep_inputs(inputs)
    res = run_bass_kernel_spmd(nc, in_maps, list(range(NCORES)))
    u = np.concatenate([res.results[c]["u_out"] for c in range(NCORES)],
                       axis=0)
    v = np.concatenate([res.results[c]["v_out"] for c in range(NCORES)],
                       axis=0)
    att_text = np.broadcast_to(u[:, None, :], (B, S, D)).astype(
        np.float32).copy()
    att_img = np.broadcast_to(v[:, None, :], (B, S, D)).astype(
        np.float32).copy()
    return att_text, att_img
